# revision 8
# baseline (speedup 1.0000x reference)
"""Trainium2 Bass kernel for nn_ATT_NLM_86320252715608 (local-attention transformer).

Data parallel: B=16 -> 2 batch items per core x 8 cores (SPMD).

The two items per core are interleaved instruction-by-instruction so that one
item's Tensor-engine work overlaps the other item's Scalar/Vector work (keeps
the PE p-state ramped and fills the scores->exp->AV serialization gaps).

Per batch item (all on device):
  - conv 7x7/49ch via im2col (49 shifted DMAs) + matmul, embed to d=128
  - residual h: feature-major bf16 [128, 3904] (64 windows x 61 tokens)
  - LN: groups of 8 122-token tiles transposed into one PSUM bank, batched
    bn_stats, per-group sqrt/recip, normalize direct from PSUM, batched
    transpose-back (affine folded into consumer weights; biases are all zero)
  - Q/K feature-major bf16 with 4 heads per tensor at partition bases
    0/32/64/96 and constant mask rows 16..20 per group (rank-1 -30
    rectangles folded into the score matmuls); V token-major per window
    pair with a ones column
  - scores S^T [122 keys, 4 groups, 244 queries] per key-window-pair into a
    2-bank PSUM tile, ONE exp per (pair, tensor) on ScalarE
  - AV: query tiles (windows 2t+1, 2t+2), 2 accumulating matmuls per head
    into token-major PSUM [122, 8*17] (softmax sums in col 16 per head)
  - normalize by 1/sums, PE-transpose back into ya (aliased with y1b)
  - O-proj + residual, FF 512 with exact gelu on ScalarE
"""

import os
import numpy as np
import ml_dtypes

BF = ml_dtypes.bfloat16

B = 16
S1 = 61
WIN = 61
S = 3721
NWPAD = 64
SPAD = NWPAD * WIN      # 3904
D = 128
H = 8
DH = 16
L = 4
FF = 512
PCH = 49
SZ = 7
KS = 3
EPS = 1e-5
CHK = 488
NCH = 8
BPB = 2
NCORE = 8
NKP = 31                # key pairs
NT = 32                 # 122-col tiles

_CACHE = {}
RUN_L = int(os.environ.get("RUN_L", str(L)))
RUN_PHASE = int(os.environ.get("RUN_PHASE", "99"))
RUN_SC = int(os.environ.get("RUN_SC", "3"))


def _head_perm():
    permA = -np.ones(128, np.int64)
    permB = -np.ones(128, np.int64)
    for h in range(4):
        permA[32 * h:32 * h + 16] = np.arange(16 * h, 16 * h + 16)
        permB[32 * h:32 * h + 16] = np.arange(64 + 16 * h, 64 + 16 * h + 16)
    return permA, permB


def _build_masks():
    wins = np.arange(SPAD) // WIN
    u = np.zeros((5, SPAD), np.float32)
    v = np.zeros((5, SPAD), np.float32)
    u[0] = np.where(wins % 4 == 0, -30., 0.); v[0] = np.where(wins % 4 == 2, 1., 0.)
    u[1] = np.where(wins % 4 == 2, -30., 0.); v[1] = np.where(wins % 4 == 0, 1., 0.)
    u[2] = np.where(wins % 4 == 1, -30., 0.); v[2] = np.where(wins % 4 == 3, 1., 0.)
    u[3] = np.where(wins % 4 == 3, -30., 0.); v[3] = np.where(wins % 4 == 1, 1., 0.)
    u[4] = np.where(wins == 61, -30., 0.)
    v[4] = np.where((wins == 59) | (wins == 60), 1., 0.)
    # rows 5..15 zero: full 16-row restore blocks
    uf = np.zeros((16, SPAD), np.float32); uf[0:5] = u
    vf = np.zeros((16, SPAD), np.float32); vf[0:5] = v
    return uf.astype(BF), vf.astype(BF)


def _sincos(n, d):
    pos = np.arange(n)[:, None].astype(np.float64)
    i = np.arange(d)[None, :]
    ang = pos / np.power(10000.0, 2 * (i // 2) / d)
    tab = np.zeros((n, d))
    tab[:, 0::2] = np.sin(ang[:, 0::2])
    tab[:, 1::2] = np.cos(ang[:, 1::2])
    return tab.astype(np.float32)


def _permw(w, perm):
    out = np.zeros_like(w)
    ok = perm >= 0
    out[:, ok] = w[:, perm[ok]]
    return out


def host_prep(ii):
    permA, permB = _head_perm()
    d = {}
    d["convwt"] = ii["conv_w"].reshape(PCH, PCH).T.copy().astype(BF)
    d["ltw"] = ii["lt_w"].astype(BF)
    posb = np.zeros((D, SPAD), np.float32)
    posb[:, :S] = _sincos(4096, D)[:S].T + ii["lt_b"][:, None]
    d["posb"] = posb.astype(BF)
    u16, v16 = _build_masks()
    d["masku"] = u16
    d["maskv"] = v16
    d["identb"] = np.eye(128, dtype=BF)
    sc = DH ** -0.5
    # all bias-like terms are zero for this problem's inputs; the device
    # program relies on that (asserted here).
    bmax = 0.0
    for i in range(L):
        s1_, b1_ = ii["ln1_s"][i], ii["ln1_b"][i]
        s2_, b2_ = ii["ln2_s"][i], ii["ln2_b"][i]
        wq = (s1_[:, None] * ii["wq"][i]) * sc
        wk = s1_[:, None] * ii["wk"][i]
        wv = s1_[:, None] * ii["wv"][i]
        for arr in (b1_ @ ii["wq"][i], b1_ @ ii["wk"][i],
                    ii["wo_b"][i] + (b1_ @ ii["wv"][i]) @ ii["wo"][i],
                    b2_ @ ii["ff_w1"][i] + ii["ff_b1"][i], ii["ff_b2"][i]):
            bmax = max(bmax, float(np.abs(arr).max()))
        d[f"wqA{i}"] = _permw(wq, permA).astype(BF)
        d[f"wqB{i}"] = _permw(wq, permB).astype(BF)
        d[f"wkA{i}"] = _permw(wk, permA).astype(BF)
        d[f"wkB{i}"] = _permw(wk, permB).astype(BF)
        d[f"wv{i}"] = wv.astype(BF)
        d[f"wo{i}"] = ii["wo"][i].astype(BF)
        d[f"w1{i}"] = (s2_[:, None] * ii["ff_w1"][i]).astype(BF)
        d[f"w2{i}"] = ii["ff_w2"][i].reshape(4, 128, 128).transpose(1, 0, 2).copy().astype(BF)
    for arr in (ii["conv_b"], ii["lt_b"], ii["pre_b1"]):
        bmax = max(bmax, float(np.abs(arr).max()))
    assert bmax < 1e-6, f"nonzero bias {bmax}; device program assumes zero biases"
    d["pw1"] = ii["pre_w1"].astype(BF)
    d["pw2"] = ii["pre_w2"].reshape(128, 1).astype(BF)
    return d


def build_program():
    import concourse.bacc as bacc
    import concourse.mybir as mybir
    import concourse.bass as bass
    from concourse.tile import TileContext
    import contextlib

    f32 = mybir.dt.float32
    bf16 = mybir.dt.bfloat16
    AF = mybir.ActivationFunctionType
    OP = mybir.AluOpType

    nc = bacc.Bacc("TRN2", target_bir_lowering=False, debug=False, num_devices=1)

    P = {}

    def dp(name, shape, dt=f32):
        P[name] = nc.declare_dram_parameter(name, list(shape), dt, isOutput=False)

    dp("x2", (BPB, S1, S1), bf16)
    dp("convwt", (PCH, PCH), bf16)
    dp("ltw", (PCH, D), bf16)
    dp("posb", (D, SPAD), bf16)
    dp("masku", (16, SPAD), bf16)
    dp("maskv", (16, SPAD), bf16)
    dp("identb", (128, 128), bf16)
    for i in range(L):
        for n in ("wqA", "wqB", "wkA", "wkB", "wv", "wo"):
            dp(f"{n}{i}", (D, D), bf16)
        dp(f"w1{i}", (D, FF), bf16)
        dp(f"w2{i}", (128, 4, 128), bf16)
    dp("pw1", (D, D), bf16)
    dp("pw2", (D, 1), bf16)
    out2 = nc.declare_dram_parameter("out2", [BPB, S1, S1], f32, isOutput=True)

    items = (0, 1)

    with TileContext(nc) as tc:
        ctx = contextlib.ExitStack()
        cons = ctx.enter_context(tc.tile_pool(name="cons", bufs=1))
        work = ctx.enter_context(tc.tile_pool(name="work", bufs=1))
        small = ctx.enter_context(tc.tile_pool(name="small", bufs=6))
        ybp = ctx.enter_context(tc.tile_pool(name="ybp", bufs=3))
        iop = ctx.enter_context(tc.tile_pool(name="iop", bufs=3))
        expp = ctx.enter_context(tc.tile_pool(name="expp", bufs=3))
        gp = ctx.enter_context(tc.tile_pool(name="gp", bufs=2))
        psb = ctx.enter_context(tc.tile_pool(name="psb", bufs=2, space="PSUM"))
        psF = ctx.enter_context(tc.tile_pool(name="psF", bufs=2, space="PSUM"))
        psH = ctx.enter_context(tc.tile_pool(name="psH", bufs=2, space="PSUM"))

        C = {}
        for name, hnd in P.items():
            if name in ("x2", "masku", "maskv", "posb"):
                continue
            t = cons.tile(list(hnd.shape), hnd.dtype, tag=f"c_{name}")
            nc.sync.dma_start(out=t[:], in_=hnd[:])
            C[name] = t
        epst = cons.tile([128, 1], f32, tag="epst")
        nc.vector.memset(epst[:], EPS)

        hA, hB, ya, QA, QB, KA, KB, Vo = {}, {}, {}, {}, {}, {}, {}, {}
        for b in items:
            hA[b] = work.tile([128, SPAD], bf16, tag=f"hA{b}", name=f"hA{b}")
            hB[b] = work.tile([128, SPAD], bf16, tag=f"hB{b}", name=f"hB{b}")
            ya[b] = work.tile([128, SPAD], bf16, tag=f"ya{b}", name=f"ya{b}")
            QA[b] = work.tile([128, SPAD], bf16, tag=f"QA{b}", name=f"QA{b}")
            QB[b] = work.tile([128, SPAD], bf16, tag=f"QB{b}", name=f"QB{b}")
            KA[b] = work.tile([128, SPAD], bf16, tag=f"KA{b}", name=f"KA{b}")
            KB[b] = work.tile([128, SPAD], bf16, tag=f"KB{b}", name=f"KB{b}")
            Vo[b] = work.tile([122, NT, 8, 17], bf16, tag=f"Vo{b}", name=f"Vo{b}")
            # softmax-denominator ones column, written once (never clobbered)
            nc.vector.memset(Vo[b][0:122, :, :, 16:17], 1.0)
        Xcol = work.tile([PCH, NWPAD, WIN], bf16, tag="Xcol")

        def ln_group(b, g, src):
            """Transpose 8 tiles of 122 tokens into one PSUM bank, batch the
            stats, normalize direct from PSUM, transpose back into ya[b]."""
            lt = psH.tile([128, 1024], bf16, tag="ph")
            base = lt[0:122, :].rearrange("p (a c) -> p a c", a=8)
            for k in range(8):
                kt = 8 * g + k
                nc.tensor.transpose(base[:, k, :], src[:, 122 * kt:122 * kt + 122],
                                    C["identb"][:])
            st = small.tile([128, 8, 6], f32, tag="st")
            for k in range(8):
                nc.vector.bn_stats(st[0:122, k, :], base[:, k, :])
            mv = small.tile([128, 8, 2], f32, tag="mv")
            for k in range(8):
                nc.vector.bn_aggr(mv[0:122, k, :], st[0:122, k, :])
            sd = small.tile([128, 8], f32, tag="sd")
            nc.scalar.activation(out=sd[0:122, :], in_=mv[0:122, :, 1], func=AF.Sqrt,
                                 bias=epst[0:122], scale=1.0)
            rs = small.tile([128, 8], f32, tag="rsg")
            nc.vector.reciprocal(rs[0:122, :], sd[0:122, :])
            yb = ybp.tile([122, 8, 128], bf16, tag="yb")
            for k in range(8):
                nc.vector.tensor_scalar(out=yb[0:122, k, :], in0=base[:, k, :],
                                        scalar1=mv[0:122, k, 0:1],
                                        scalar2=rs[0:122, k:k + 1],
                                        op0=OP.subtract, op1=OP.mult)
            for half in range(2):
                bk = psH.tile([128, 1024], bf16, tag="ph")
                bkv = bk[0:128, 0:488].rearrange("p (a c) -> p a c", a=4)
                for kk in range(4):
                    nc.tensor.transpose(bkv[:, kk, 0:122], yb[0:122, 4 * half + kk, :],
                                        C["identb"][0:122, 0:122])
                nc.vector.tensor_copy(ya[b][:, 976 * g + 488 * half:976 * g + 488 * half + 488],
                                      bk[0:128, 0:488])

        def projqk(b, j, li):
            c0 = j * CHK
            for dst, wname, eng in ((QA, f"wqA{li}", 0), (QB, f"wqB{li}", 1),
                                    (KA, f"wkA{li}", 0), (KB, f"wkB{li}", 1)):
                ps = psF.tile([128, 512], f32, tag="pf")
                nc.tensor.matmul(ps[:, 0:CHK], C[wname][:], ya[b][:, c0:c0 + CHK],
                                 start=True, stop=True)
                if eng == 0:
                    nc.scalar.activation(out=dst[b][:, c0:c0 + CHK],
                                         in_=ps[:, 0:CHK], func=AF.Copy)
                else:
                    nc.vector.tensor_copy(dst[b][:, c0:c0 + CHK], ps[:, 0:CHK])

        def write_masks(b):
            for g in range(4):
                for t in (KA[b], KB[b]):
                    nc.sync.dma_start(out=t[32 * g + 16:32 * g + 32, :], in_=P["masku"][:])
                for t in (QA[b], QB[b]):
                    nc.sync.dma_start(out=t[32 * g + 16:32 * g + 32, :], in_=P["maskv"][:])

        def vproj(b, p, li):
            c0 = p * 122
            ps = psF.tile([128, 512], f32, tag="pf")
            nc.tensor.matmul(ps[0:122, 0:128], ya[b][:, c0:c0 + 122], C[f"wv{li}"][:],
                             start=True, stop=True)
            nc.vector.tensor_copy(Vo[b][0:122, p, :, 0:16],
                                  ps[0:122, 0:128].rearrange("p (h j) -> p h j", h=8))

        def scores_exp(b, c, ex_tiles):
            k0 = c * 122
            q0 = max(0, 2 * c - 1) * WIN
            ex = expp.tile([122, 2, 4, 244], bf16, tag=f"ex{b}")
            for tens, (QT, KT) in enumerate(((QA[b], KA[b]), (QB[b], KB[b]))):
                ps = psb.tile([128, 4, 256], f32, tag="sc")
                ng = 1 if RUN_SC == 1 else 4
                for g in range(ng):
                    nc.tensor.matmul(ps[0:122, g, 0:244],
                                     KT[32 * g:32 * g + 21, k0:k0 + 122],
                                     QT[32 * g:32 * g + 21, q0:q0 + 244],
                                     start=True, stop=True,
                                     tile_position=(32 * g, 0))
                if RUN_SC >= 3:
                    nc.scalar.activation(out=ex[:, tens, :, :], in_=ps[0:122, :, 0:244],
                                         func=AF.Exp)
            ex_tiles[c] = (ex, q0)

        def avtile(b, t, ex_tiles):
            qc0 = 0 if t < 0 else 61 + 122 * t
            M = 61 if t < 0 else 122
            pf = psF.tile([128, 512], f32, tag="pf")
            av = pf[0:122, 0:136]
            avv = av.rearrange("p (h j) -> p h j", h=8)
            contribs = [c for c in (t, t + 1) if 0 <= c <= 30]
            for hh in range(8):
                tens, g = hh // 4, hh % 4
                for ci, c in enumerate(contribs):
                    ex, q0 = ex_tiles[c]
                    lo = qc0 - q0
                    nc.tensor.matmul(av[0:M, 17 * hh:17 * hh + 17],
                                     ex[:, tens, g, lo:lo + M],
                                     Vo[b][:, c, hh, :],
                                     start=(ci == 0), stop=(ci == len(contribs) - 1))
            rs = small.tile([122, 8, 1], f32, tag="avrs")
            nc.vector.reciprocal(rs[0:M, :, :], avv[0:M, :, 16:17])
            On = small.tile([122, 8, 16], bf16, tag="On")
            rs_b = bass.AP(tensor=rs.tensor, offset=rs[0:M, :, :].offset,
                           ap=[[rs.ap[0][0], M], [rs.ap[1][0], 8], [0, 16]])
            nc.vector.tensor_tensor(out=On[0:M, :, :], in0=avv[0:M, :, 0:16],
                                    in1=rs_b, op=OP.mult)
            pt = psH.tile([128, 1024], bf16, tag="ph")
            nc.tensor.transpose(pt[0:128, 0:M],
                                On[0:M, :, :].rearrange("p h j -> p (h j)"),
                                C["identb"][0:M, 0:M])
            nc.vector.tensor_copy(ya[b][:, qc0:qc0 + M], pt[0:128, 0:M])

        def oproj(b, j, li, hin, hout):
            c0 = j * CHK
            ps = psF.tile([128, 512], f32, tag="pf")
            nc.tensor.matmul(ps[:, 0:CHK], C[f"wo{li}"][:], ya[b][:, c0:c0 + CHK],
                             start=True, stop=True)
            nc.vector.tensor_tensor(out=hout[:, c0:c0 + CHK], in0=ps[:, 0:CHK],
                                    in1=hin[:, c0:c0 + CHK], op=OP.add)

        def ff1(b, q, li, Gcur):
            Gt = gp.tile([128, 4, 976], bf16, tag="G")
            Gcur[b] = Gt
            t0 = q * 976
            for s in range(4):
                ps = psb.tile([128, 4, 256], f32, tag="sc")
                for sub in range(2):
                    j0 = t0 + sub * CHK
                    nc.tensor.matmul(ps[:, 2 * sub:2 * sub + 2, 0:244],
                                     C[f"w1{li}"][:, s * 128:(s + 1) * 128],
                                     ya[b][:, j0:j0 + CHK], start=True, stop=True)
                nc.scalar.activation(out=Gt[:, s, :].rearrange("p (a c) -> p a c", a=4),
                                     in_=ps[:, :, 0:244], func=AF.Gelu)

        def ff2(b, q, li, Gcur, hout):
            for jj in range(2):
                j = q * 2 + jj
                c0 = j * CHK
                ps = psF.tile([128, 512], f32, tag="pf")
                for s in range(4):
                    nc.tensor.matmul(ps[:, 0:CHK], C[f"w2{li}"][:, s, :],
                                     Gcur[b][:, s, jj * CHK:(jj + 1) * CHK],
                                     start=(s == 0), stop=(s == 3))
                nc.vector.tensor_tensor(out=hout[:, c0:c0 + CHK], in0=ps[:, 0:CHK],
                                        in1=hout[:, c0:c0 + CHK], op=OP.add)

        def conv_embed(b, hdst):
            nc.vector.memset(Xcol[:], 0.0)
            for dy in range(SZ):
                for dx in range(SZ):
                    k = dy * SZ + dx
                    oy, ox = dy - KS, dx - KS
                    iy0, iy1 = max(0, -oy), min(S1, S1 - oy)
                    ix0, ix1 = max(0, -ox), min(S1, S1 - ox)
                    nc.sync.dma_start(
                        out=Xcol[k:k + 1, iy0:iy1, ix0:ix1],
                        in_=P["x2"][b, iy0 + oy:iy1 + oy, ix0 + ox:ix1 + ox])
            Xf = Xcol[:].rearrange("p r c -> p (r c)")
            for j in range(NCH):
                c0 = j * CHK
                ps = psF.tile([128, 512], f32, tag="pf")
                nc.tensor.matmul(ps[0:PCH, 0:CHK], C["convwt"][:], Xf[:, c0:c0 + CHK],
                                 start=True, stop=True)
                pc = iop.tile([PCH, CHK], bf16, tag="pc")
                nc.scalar.activation(out=pc[:], in_=ps[0:PCH, 0:CHK], func=AF.Relu)
                ps2 = psF.tile([128, 512], f32, tag="pf")
                nc.tensor.matmul(ps2[:, 0:CHK], C["ltw"][:], pc[:],
                                 start=True, stop=True)
                pb = iop.tile([128, CHK], bf16, tag="pb")
                nc.sync.dma_start(out=pb[:], in_=P["posb"][:, c0:c0 + CHK])
                nc.vector.tensor_tensor(out=hdst[:, c0:c0 + CHK], in0=ps2[:, 0:CHK],
                                        in1=pb[:], op=OP.add)

        def head(b, hin):
            for j in range(NCH):
                c0 = j * CHK
                ps = psF.tile([128, 512], f32, tag="pf")
                nc.tensor.matmul(ps[:, 0:CHK], C["pw1"][:], hin[:, c0:c0 + CHK],
                                 start=True, stop=True)
                nc.scalar.activation(out=KA[b][:, c0:c0 + CHK], in_=ps[:, 0:CHK],
                                     func=AF.Relu)
            for j in range(NCH):
                c0 = j * CHK
                ps = psF.tile([128, 512], f32, tag="pf")
                nc.tensor.matmul(ps[0:1, 0:CHK], C["pw2"][:], KA[b][:, c0:c0 + CHK],
                                 start=True, stop=True)
                oc = iop.tile([1, CHK], f32, tag="oc")
                nc.vector.tensor_copy(oc[:], ps[0:1, 0:CHK])
                r0 = j * 8
                nr = min(8, S1 - r0)
                if nr > 0:
                    nc.sync.dma_start(
                        out=out2[b:b + 1, r0:r0 + nr, :],
                        in_=oc[:, 0:nr * S1].rearrange("p (r c) -> p r c", r=nr))

        for b in items:
            conv_embed(b, hA[b])
        for li in range(RUN_L):
            hin = {b: (hA[b] if li % 2 == 0 else hB[b]) for b in items}
            hout = {b: (hB[b] if li % 2 == 0 else hA[b]) for b in items}
            def bail():
                for b in items:
                    nc.vector.tensor_copy(hout[b][:], hin[b][:])
            for g in range(4):
                for b in items:
                    ln_group(b, g, hin[b])
            if RUN_PHASE < 1:
                bail(); continue
            for j in range(NCH):
                for b in items:
                    projqk(b, j, li)
            if RUN_PHASE < 2:
                bail(); continue
            for b in items:
                write_masks(b)
            if RUN_PHASE < 3:
                bail(); continue
            for p in range(NT):
                for b in items:
                    vproj(b, p, li)
            if RUN_PHASE < 4:
                bail(); continue
            ex_tiles = {b: {} for b in items}
            for c in range(NKP):
                for b in items:
                    scores_exp(b, c, ex_tiles[b])
                if RUN_PHASE >= 5:
                    for b in items:
                        avtile(b, c - 1, ex_tiles[b])
            if RUN_PHASE < 6:
                bail(); continue
            for j in range(NCH):
                for b in items:
                    oproj(b, j, li, hin[b], hout[b])
            if RUN_PHASE < 7:
                continue
            for g in range(4):
                for b in items:
                    ln_group(b, g, hout[b])
            Gcur = {}
            for q in range(4):
                for b in items:
                    ff1(b, q, li, Gcur)
                for b in items:
                    ff2(b, q, li, Gcur, hout[b])
        for b in items:
            head(b, hA[b] if RUN_L % 2 == 0 else hB[b])

        ctx.close()
    nc.compile()
    return nc


def kernel(**inputs):
    from concourse.bass_utils import run_bass_kernel_spmd

    ii = {k: np.asarray(v) for k, v in inputs.items()}
    d = host_prep(ii)
    pb2 = float(ii["pre_b2"].reshape(-1)[0])
    if "prog" not in _CACHE:
        _CACHE["prog"] = build_program()
    nc = _CACHE["prog"]

    xb = ii["x"].astype(BF)            # [16, 61, 61]
    in_maps = []
    for core in range(NCORE):
        m = dict(d)
        m["x2"] = xb[core * BPB:(core + 1) * BPB]
        in_maps.append(m)
    res = run_bass_kernel_spmd(nc, in_maps, core_ids=list(range(NCORE)))
    _CACHE["last_res"] = res
    out = np.concatenate([res.results[i]["out2"] for i in range(NCORE)], axis=0)
    return (out + pb2).astype(np.float32)
